# revision 2
# baseline (speedup 1.0000x reference)
"""Trainium2 Bass kernel for nn_Encoder_66735201845341.

Computes h = sum_rows(x @ W.T) for x [500000, 256] f32, W [128, 256] f32,
returning [1, 128] f32.

Strategy (8 NeuronCores, data-parallel over rows of x):
  - Host: cast x to bf16 (the 2e-2 tolerance leaves 10x headroom over the
    ~1.7e-3 cast error) and shard row-wise into 8 equal shards (62500 rows),
    zero-padded to 62592 rows (489*128) so each shard reshapes to
    [128, 125184] with every SBUF partition holding whole 256-element rows.
  - Device (per core): stream the shard through SBUF in [128, 8192] bf16
    tiles (2 MiB DMAs, saturating the ~360+ GB/s per-core HBM bandwidth
    from a single HWDGE ring), and column-sum on the Tensor engine with
    ones-matmuls into a single fp32 PSUM accumulator: psum[0, j] +=
    sum_p tile[p, j'] for 512-wide slices, where j' mod 256 == j mod 256
    by construction. PE runs at ~240 ns per 512-slice and hides entirely
    under the DMA stream; PSUM accumulation is exact fp32.
  - Tail: fold the [1, 512] column sum to [128, 2] via K=1 transpose
    matmuls, project through W.T (host-pretransposed) with two
    K=128-contraction matmuls -> partial h [1, 128] per core.
  - Host: gather the 8 row-shard partials and sum them (the unshard step
    for a sum-sharded output); no device collective needed.
"""

import numpy as np

N_CORES = 8
ROWS = 500000
COLS = 256
OUT = 128
P = 128
ROWS_PER_CORE = ROWS // N_CORES  # 62500
PAD_ROWS = 62592  # 489 * 128
FREE = PAD_ROWS * COLS // P  # 125184 bf16 per partition
F = 8192  # 2 MiB bf16 DMA tiles
NS = 512  # moving-operand slice per matmul (one fp32 PSUM bank)

_CACHE = {}


def _build(
    use_collective=False,
    repeat=1,
    num_devices=N_CORES,
    tail_repeat=1,
    f_tile=F,
    bufs=4,
):
    """bf16-input column-sum kernel. x is cast to bf16 on the host (halving
    HBM read traffic); the column sum runs on the Tensor engine as
    ones-matmuls with exact fp32 accumulation in PSUM, so the only precision
    loss is the one-time fp32->bf16 cast of x (~1.7e-3 rel err on the
    output). repeat/tail_repeat are timing-only knobs that repeat the bulk
    pass / the tail inside one NEFF for wall-clock slope measurement.
    """
    import concourse.bacc as bacc
    import concourse.mybir as mybir
    from concourse.tile import TileContext

    dt = mybir.dt.float32
    db = mybir.dt.bfloat16
    nc = bacc.Bacc(
        "TRN2", target_bir_lowering=False, debug=False, num_devices=num_devices
    )
    xs = nc.dram_tensor("xs", [P, FREE], db, kind="ExternalInput")
    wt = nc.dram_tensor("wt", [COLS, OUT], dt, kind="ExternalInput")
    y = nc.dram_tensor("y", [1, OUT], dt, kind="ExternalOutput")

    # Taper the last tiles so each tile's PE matmuls (which wait for the
    # whole tile's DMA) hide under the next tile's DMA; under a microsecond
    # of PE work remains after the final DMA lands. Non-final widths stay
    # multiples of 512 so every slice maps to PSUM position j mod 256.
    TAIL = [4096, 2560, 2048, 1792]
    offs = []
    o = 0
    while o < FREE - sum(TAIL):
        f = min(f_tile, FREE - sum(TAIL) - o)
        offs.append((o, f))
        o += f
    for f in TAIL:
        offs.append((o, f))
        o += f
    assert o == FREE

    with TileContext(nc) as tc:
        with (
            tc.tile_pool(name="xt", bufs=bufs) as xpool,
            tc.tile_pool(name="work", bufs=1) as wpool,
            tc.tile_pool(name="psum", bufs=1, space="PSUM") as ppool,
            tc.tile_pool(name="dram", bufs=1, space="DRAM") as dpool,
        ):
            # Weight loads go on the scalar HWDGE ring so they don't delay
            # the first x-tile DMA on the sync ring.
            wt0 = wpool.tile([P, OUT], dt, tag="wt0")
            wt1 = wpool.tile([P, OUT], dt, tag="wt1")
            nc.scalar.dma_start(wt0[:], wt[0:P, :])
            nc.scalar.dma_start(wt1[:], wt[P:COLS, :])
            ones = wpool.tile([P, 1], db, tag="ones")
            nc.vector.memset(ones[:], 1.0)
            ones1 = wpool.tile([1, 1], dt, tag="ones1")
            nc.vector.memset(ones1[:], 1.0)

            # Column-sum accumulator: psum_cs[0, j] += sum_p xt[p, j'] for
            # every slice; j' mod 256 == j mod 256 by construction.
            psum_cs = ppool.tile([1, NS], dt, tag="csum")
            n_slices = repeat * sum(-(-f // NS) for _, f in offs)
            k = 0
            for _rep in range(repeat):
                for o, f in offs:
                    xt = xpool.tile([P, f_tile], db, tag="xt")
                    nc.sync.dma_start(xt[:, :f], xs[:, o : o + f])
                    for s in range(0, f, NS):
                        sl = min(NS, f - s)
                        k += 1
                        nc.tensor.matmul(
                            psum_cs[0:1, 0:sl],
                            ones[:],
                            xt[:, s : s + sl],
                            start=k == 1,
                            stop=k == n_slices,
                            skip_group_check=True,
                        )

            for _tail_rep in range(tail_repeat):
                cs_sb = wpool.tile([1, NS], dt, tag="cs_sb")
                nc.vector.tensor_copy(cs_sb[:], psum_cs[:])
                # Transpose the 1-partition column sum into [128, 2] via
                # K=1 matmuls, folding the two 256-halves of each column.
                # One PSUM tile (bank) per accumulation group — interleaved
                # groups in one bank corrupt the first group's partial.
                pms = [
                    ppool.tile([P, 1], dt, tag=f"pm{h}", name=f"pm{h}")
                    for h in range(2)
                ]
                for h in range(2):
                    nc.tensor.matmul(
                        pms[h][:],
                        cs_sb[0:1, h * 128 : (h + 1) * 128],
                        ones1[:],
                        start=True,
                        stop=False,
                    )
                    nc.tensor.matmul(
                        pms[h][:],
                        cs_sb[0:1, (h + 2) * 128 : (h + 3) * 128],
                        ones1[:],
                        start=False,
                        stop=True,
                    )
                cb = wpool.tile([P, 2], dt, tag="csb")
                nc.vector.tensor_copy(cb[:, 0:1], pms[0][:])
                nc.vector.tensor_copy(cb[:, 1:2], pms[1][:])
                hp = ppool.tile([1, OUT], dt, tag="h")
                nc.tensor.matmul(hp[:], cb[:, 0:1], wt0[:], start=True, stop=False)
                nc.tensor.matmul(hp[:], cb[:, 1:2], wt1[:], start=False, stop=True)
                hs = wpool.tile([1, OUT], dt, tag="hs")
                nc.vector.tensor_copy(hs[:], hp[:])
                if use_collective:
                    import concourse.mybir as _mybir

                    ib = dpool.tile([1, OUT], dt, tag="ib")
                    ob = dpool.tile([1, OUT], dt, tag="ob")
                    nc.sync.dma_start(ib[:], hs[:])
                    nc.gpsimd.collective_compute(
                        "AllReduce",
                        _mybir.AluOpType.add,
                        replica_groups=[list(range(N_CORES))],
                        ins=[ib.opt()],
                        outs=[ob.opt()],
                    )
                    nc.sync.dma_start(y[:], ob[:])
                else:
                    nc.sync.dma_start(y[:], hs[:])
    nc.compile()
    return nc


def _get_nc():
    key = "nc"
    if key not in _CACHE:
        _CACHE[key] = _build()
    return _CACHE[key]


def make_in_maps(x, W):
    import ml_dtypes

    x = np.asarray(x, dtype=np.float32)
    W = np.asarray(W, dtype=np.float32)
    wt = np.ascontiguousarray(W.T)  # [256, 128]
    in_maps = []
    for c in range(N_CORES):
        shard = np.zeros((PAD_ROWS, COLS), dtype=ml_dtypes.bfloat16)
        shard[:ROWS_PER_CORE] = x[c * ROWS_PER_CORE : (c + 1) * ROWS_PER_CORE]
        in_maps.append({"xs": shard.reshape(P, FREE), "wt": wt})
    return in_maps


def kernel(x, W):
    from concourse.bass_utils import run_bass_kernel_spmd

    nc = _get_nc()
    in_maps = make_in_maps(x, W)
    out = None
    for attempt in range(3):
        try:
            res = run_bass_kernel_spmd(nc, in_maps, core_ids=list(range(N_CORES)))
        except Exception:
            if attempt == 2:
                raise
            continue
        ys = [r["y"] for r in res.results]
        # Unshard: the output is sum-sharded over the row shards, so the
        # gather step is summing the 8 per-core partials.
        out = np.sum(np.stack(ys, axis=0), axis=0, dtype=np.float64).astype(
            np.float32
        )
        # An all-zero partial for nonzero input indicates a transient
        # execution failure (PJRT returns the donated zero buffer) — retry.
        if all(np.any(yc) for yc in ys):
            return out
    return out
